# revision 16
# baseline (speedup 1.0000x reference)
"""Trainium2 Bass kernel for a pre-LN transformer decoder block.

Problem: x:[2,2048,1024] f32, causal mask, 16 heads, DFF=4096.
  out = x + Attn(LN1(x)); out = out + FFN(LN2(out))

Strategy (8 NeuronCores, collective-free SPMD):
  - Core c handles batch c//4, query rows [(c%4)*512, +512).
  - Each core redundantly computes LN1 + K/V over its batch's full 2048
    context (uniform program; per-core differences live only in input
    data).
  - Context PERMUTATION trick: the host reorders each core's context
    chunks so the core's own (diagonal) 512-chunk sits at fixed position
    [1536, 2048). Causal masking then needs only (a) a 512x512 diagonal
    mask applied to the last 4 key tiles, and (b) a per-key "alive"
    column (vzero) folded into V staging, which zeroes both the V rows
    and the softmax-denominator ones-column of dead keys. Attention sums
    are key-order invariant, so results are exact.
  - Everything stays in SBUF: Q/K/V, attention outputs; weights are
    streamed or prefetched from HBM under compute.
  - Softmax without max-subtraction (scores provably small); row sums
    via a ones-column appended to V; exp batched 2 PSUM banks per
    activation instruction.
  - Matmuls bf16 (f32 PSUM accumulate); LN stats in f32/bf16 mix.
"""

import sys
import contextlib
import numpy as np

for _p in ("/opt/trn_rl_repo", "/root/.axon_site/_ro/trn_rl_repo"):
    if _p not in sys.path:
        sys.path.insert(0, _p)

import ml_dtypes  # noqa: E402
import concourse.bass as bass  # noqa: E402
import concourse.mybir as mybir  # noqa: E402
import concourse.tile as tile  # noqa: E402
from concourse import bacc  # noqa: E402
from concourse.bass_utils import run_bass_kernel_spmd  # noqa: E402
from concourse.masks import make_identity  # noqa: E402

P = 128
DH = 64
EPS = 1e-5
BF16 = mybir.dt.bfloat16
F32 = mybir.dt.float32
AF = mybir.ActivationFunctionType

_PROG_CACHE = {}


def _build_program(S, D, H, DFF, TQ, n_iter=1,
                   phases=frozenset({'lnq', 'qkv', 'attn', 'wo', 'ffn'})):
    """One-core SPMD program: decoder block for TQ=512 query rows with a
    (permuted) S=2048-row context. All cores run this identical program
    on different data."""
    J = D // P            # 8 feature tiles
    JF = DFF // P         # 32
    KC = S // P           # 16 key tiles
    NT = S // TQ          # 4 context chunks of 512
    VW = 2 * (DH + 1)     # v_sb cols per head-pair tile (130)
    assert TQ == 512 and H == 2 * J

    nc = bacc.Bacc(None, target_bir_lowering=False)

    # ---- I/O ----
    xcT = nc.dram_tensor("xcT", [D, S], BF16, kind="ExternalInput")
    xqT = nc.dram_tensor("xqT", [D, TQ], F32, kind="ExternalInput")
    maskT = nc.dram_tensor("maskT", [TQ, TQ], BF16, kind="ExternalInput")
    vzero = nc.dram_tensor("vzero", [S], F32, kind="ExternalInput")
    wqT = nc.dram_tensor("wqT", [D, D], BF16, kind="ExternalInput")
    wkT = nc.dram_tensor("wkT", [D, D], BF16, kind="ExternalInput")
    wvT = nc.dram_tensor("wvT", [D, D], BF16, kind="ExternalInput")
    woT = nc.dram_tensor("woT", [D, D], BF16, kind="ExternalInput")
    w1T = nc.dram_tensor("w1T", [D, DFF], BF16, kind="ExternalInput")
    w2T = nc.dram_tensor("w2T", [DFF, D], BF16, kind="ExternalInput")
    ln1g = nc.dram_tensor("ln1g", [D], F32, kind="ExternalInput")
    ln1b = nc.dram_tensor("ln1b", [D], F32, kind="ExternalInput")
    ln2g = nc.dram_tensor("ln2g", [D], F32, kind="ExternalInput")
    ln2b = nc.dram_tensor("ln2b", [D], F32, kind="ExternalInput")
    b1 = nc.dram_tensor("b1", [DFF], F32, kind="ExternalInput")
    b2 = nc.dram_tensor("b2", [D], F32, kind="ExternalInput")
    outT = nc.dram_tensor("outT", [D, TQ], F32, kind="ExternalOutput")

    xcT_r = xcT.rearrange("(j p) t -> p j t", p=P)
    xqT_r = xqT.rearrange("(j p) t -> p j t", p=P)
    maskT_r = maskT.rearrange("(kc p) q -> p kc q", p=P)
    vz_r = vzero.rearrange("(kc p) -> p kc", p=P)
    wqT_r = wqT.rearrange("(j p) e -> p j e", p=P)
    wkT_r = wkT.rearrange("(j p) e -> p j e", p=P)
    wvT_r = wvT.rearrange("(j p) e -> p j e", p=P)
    woT_r = woT.rearrange("(j p) e -> p j e", p=P)
    w1T_r = w1T.rearrange("(j p) f -> p j f", p=P)
    w2T_r = w2T.rearrange("(jf p) e -> p jf e", p=P)
    outT_r = outT.rearrange("(j p) q -> p j q", p=P)

    loop_cm = nc.Fori(0, n_iter) if n_iter > 1 else contextlib.nullcontext()
    with loop_cm, tile.TileContext(nc) as tc:
        with (
            tc.tile_pool(name="const", bufs=1) as const,
            tc.tile_pool(name="persist", bufs=1) as persist,
            tc.tile_pool(name="scr", bufs=2) as scr,
            tc.tile_pool(name="scr_s", bufs=2) as scr_s,
            tc.tile_pool(name="ps_small", bufs=2, space="PSUM") as ps_small,
        ):
            # constants
            ones_col = const.tile([P, 1], BF16)
            nc.vector.memset(ones_col[:], 1.0)
            ones_row = const.tile([1, P], BF16)
            nc.vector.memset(ones_row[:], 1.0)
            eps_t = const.tile([1, 1], F32)
            nc.vector.memset(eps_t[:], EPS)
            ident = const.tile([P, P], BF16)
            make_identity(nc, ident[:])
            ln1g_c = const.tile([P, J], F32)
            nc.sync.dma_start(ln1g_c[:], ln1g.rearrange("(j p) -> p j", p=P))
            ln1b_c = const.tile([P, J], F32)
            nc.sync.dma_start(ln1b_c[:], ln1b.rearrange("(j p) -> p j", p=P))
            ln2g_c = const.tile([P, J], F32)
            nc.sync.dma_start(ln2g_c[:], ln2g.rearrange("(j p) -> p j", p=P))
            ln2b_c = const.tile([P, J], F32)
            nc.sync.dma_start(ln2b_c[:], ln2b.rearrange("(j p) -> p j", p=P))
            b1_c = const.tile([P, JF], F32)
            nc.sync.dma_start(b1_c[:], b1.rearrange("(j p) -> p j", p=P))
            b2_c = const.tile([P, J], F32)
            nc.sync.dma_start(b2_c[:], b2.rearrange("(j p) -> p j", p=P))
            vz_c = const.tile([P, KC], F32)
            nc.sync.dma_start(vz_c[:], vz_r)
            vzb_c = const.tile([P, KC], BF16)
            nc.vector.tensor_copy(vzb_c[:], vz_c[:])

            # persistent activations
            xq_sb = persist.tile([P, J, TQ], F32)
            nc.sync.dma_start(xq_sb[:], xqT_r)
            y_sb = persist.tile([P, J, TQ], F32)
            attn_sb = persist.tile([P, J, TQ], BF16)
            wo_w = persist.tile([P, J, D], BF16)

            def layer_norm(src_sb, g_c, b_c, out_sb, src_f32):
                """Column LN over J*P features: src [P,J,TQ] -> out bf16."""
                ps_x = ps_small.tile([P, TQ], F32, tag="misc")
                ps_q = ps_small.tile([P, TQ], F32, tag="misc")
                xbs = []
                for j in range(J):
                    if src_f32:
                        xb = scr.tile([P, TQ], BF16, tag="xb")
                        nc.vector.tensor_copy(xb[:], src_sb[:, j, :])
                        xb = xb[:]
                    else:
                        xb = src_sb[:, j, :]
                    xbs.append(xb)
                    sq = scr.tile([P, TQ], BF16, tag="sq")
                    nc.vector.tensor_mul(sq[:], xb, xb)
                    nc.tensor.matmul(ps_x[:1, :], ones_col[:], xb,
                                     start=(j == 0), stop=(j == J - 1))
                    nc.tensor.matmul(ps_q[:1, :], ones_col[:], sq[:],
                                     start=(j == 0), stop=(j == J - 1))
                inv_d = 1.0 / D
                mu = scr_s.tile([1, TQ], F32, tag="mu", bufs=1)
                nc.scalar.mul(mu[:], ps_x[:1, :], inv_d)
                ex2 = scr_s.tile([1, TQ], F32, tag="ex2", bufs=1)
                nc.scalar.mul(ex2[:], ps_q[:1, :], inv_d)
                var = scr_s.tile([1, TQ], F32, tag="var", bufs=1)
                nc.vector.tensor_mul(var[:], mu[:], mu[:])
                nc.vector.tensor_sub(var[:], ex2[:], var[:])
                nc.scalar.activation(var[:], var[:], AF.Ln,
                                     bias=eps_t[:], scale=1.0)
                nc.scalar.activation(var[:], var[:], AF.Exp,
                                     bias=0.0, scale=-0.5)
                mub = scr_s.tile([1, TQ], BF16, tag="mub")
                nc.scalar.copy(mub[:], mu[:])
                rsb = scr_s.tile([1, TQ], BF16, tag="rsb")
                nc.scalar.copy(rsb[:], var[:])
                pmu = ps_small.tile([P, TQ], F32, tag="misc")
                nc.tensor.matmul(pmu[:], ones_row[:], mub[:],
                                 start=True, stop=True)
                prs = ps_small.tile([P, TQ], F32, tag="misc")
                nc.tensor.matmul(prs[:], ones_row[:], rsb[:],
                                 start=True, stop=True)
                for j in range(J):
                    t1 = scr.tile([P, TQ], F32, tag="t1")
                    nc.vector.tensor_sub(t1[:], src_sb[:, j, :], pmu[:])
                    nc.vector.tensor_mul(t1[:], t1[:], prs[:])
                    nc.scalar.activation(out_sb[:, j, :], t1[:], AF.Identity,
                                         bias=b_c[:, j:j + 1],
                                         scale=g_c[:, j:j + 1])

            # attention-data pool (lives through attention)
            with tc.tile_pool(name="attd", bufs=1) as attd:
                q_sb = attd.tile([P, J, TQ], BF16)
                k_sb = attd.tile([P, J, S], BF16)
                v_sb = attd.tile([P, KC, J * VW], BF16)
                mask_sb = attd.tile([P, 4, TQ], BF16)
                nc.sync.dma_start(mask_sb[:], maskT_r)
                # ones columns (killed later for dead keys by vzb values)
                for hh in range(H):
                    c0 = (hh // 2) * VW + (hh % 2) * (DH + 1) + DH
                    nc.vector.tensor_copy(v_sb[:, :, c0], vzb_c[:])

                # ---------- Phase A: LN1 + Q/K/V projections ----------
                with tc.tile_pool(name="sA", bufs=1) as sA, \
                     tc.tile_pool(name="sAw", bufs=3) as sAw, \
                     tc.tile_pool(name="sAl", bufs=2) as sAl, \
                     tc.tile_pool(name="ps_mm", bufs=2, space="PSUM") as ps_mm, \
                     tc.tile_pool(name="ps_tr", bufs=2, space="PSUM") as ps_tr:
                    ln1_own = sA.tile([P, J, TQ], BF16)
                    if 'lnq' in phases:
                        layer_norm(xq_sb, ln1g_c, ln1b_c, ln1_own, True)
                    for m in range(J if 'lnq' in phases else 0):
                        ms = slice(m * P, (m + 1) * P)
                        wt = sAw.tile([P, J, P], BF16, tag="w8")
                        nc.sync.dma_start(wt[:], wqT_r[:, :, ms])
                        ps = ps_mm.tile([P, TQ], F32, tag="mm")
                        for j in range(J):
                            nc.tensor.matmul(ps[:], wt[:, j, :],
                                             ln1_own[:, j, :],
                                             start=(j == 0), stop=(j == J - 1))
                        nc.scalar.copy(q_sb[:, m, :], ps[:])

                    for t in range(NT if 'qkv' in phases else 0):
                        ts = slice(t * TQ, (t + 1) * TQ)
                        xct = sAl.tile([P, J, TQ], BF16, tag="xct", bufs=1)
                        nc.sync.dma_start(xct[:], xcT_r[:, :, ts])
                        ln1_t = sAl.tile([P, J, TQ], BF16, tag="ln1")
                        layer_norm(xct, ln1g_c, ln1b_c, ln1_t, False)
                        for m in range(J):
                            ms = slice(m * P, (m + 1) * P)
                            wt = sAw.tile([P, J, P], BF16, tag="w8")
                            nc.sync.dma_start(wt[:], wkT_r[:, :, ms])
                            ps = ps_mm.tile([P, TQ], F32, tag="mm")
                            for j in range(J):
                                nc.tensor.matmul(ps[:], wt[:, j, :],
                                                 ln1_t[:, j, :],
                                                 start=(j == 0),
                                                 stop=(j == J - 1))
                            nc.scalar.copy(k_sb[:, m, ts], ps[:])

                            wt = sAw.tile([P, J, P], BF16, tag="w8")
                            nc.sync.dma_start(wt[:], wvT_r[:, :, ms])
                            ps = ps_mm.tile([P, TQ], F32, tag="mm")
                            for j in range(J):
                                nc.tensor.matmul(ps[:], wt[:, j, :],
                                                 ln1_t[:, j, :],
                                                 start=(j == 0),
                                                 stop=(j == J - 1))
                            vst = scr.tile([P, TQ], BF16, tag="vst")
                            nc.scalar.copy(vst[:], ps[:])
                            for kt in range(4):
                                kc = t * 4 + kt
                                ps_t = ps_tr.tile([P, P], BF16, tag="tr")
                                nc.tensor.transpose(
                                    ps_t[:],
                                    vst[:, kt * P:(kt + 1) * P], ident[:])
                                vb0 = m * VW
                                nc.vector.tensor_scalar_mul(
                                    v_sb[:, kc, vb0:vb0 + DH],
                                    ps_t[:, 0:DH], vz_c[:, kc:kc + 1])
                                nc.vector.tensor_scalar_mul(
                                    v_sb[:, kc, vb0 + DH + 1:vb0 + 2 * DH + 1],
                                    ps_t[:, DH:2 * DH], vz_c[:, kc:kc + 1])

                # ---------- Phase B: attention ----------
                nc.sync.dma_start(wo_w[:], woT_r)
                with tc.tile_pool(name="sBe", bufs=3) as sBe, \
                     tc.tile_pool(name="ps_sc", bufs=2, space="PSUM") as ps_sc, \
                     tc.tile_pool(name="ps_av", bufs=2, space="PSUM") as ps_av:
                    for h in range(H if 'attn' in phases else 0):
                        m, lo = h // 2, (h % 2) * DH
                        vb = m * VW + (h % 2) * (DH + 1)
                        ob, dr = 0, DH
                        qr = q_sb[lo:lo + DH, m, :]
                        pav = ps_av.tile([P, TQ], F32, tag="av")
                        pend = []
                        for kp in range(KC // 2):
                            ps2 = ps_sc.tile([P, 2, TQ], F32, tag="sc")
                            for hf in range(2):
                                kc = 2 * kp + hf
                                nc.tensor.matmul(
                                    ps2[:, hf, :],
                                    k_sb[lo:lo + DH, m, kc * P:(kc + 1) * P],
                                    qr, start=True, stop=True)
                            ex2 = sBe.tile([P, 2, TQ], BF16, tag="ex")
                            nc.scalar.activation(ex2[:], ps2[:], AF.Exp,
                                                 scale=1.0 / np.sqrt(DH))
                            for hf in range(2):
                                kc = 2 * kp + hf
                                if kc >= KC - 4:
                                    nc.vector.tensor_mul(
                                        ex2[:, hf, :], ex2[:, hf, :],
                                        mask_sb[:, kc - (KC - 4), :])
                                pend.append((kc, ex2))
                            while len(pend) > 2:
                                kc0, e0 = pend.pop(0)
                                nc.tensor.matmul(
                                    pav[ob:ob + DH + 1, :],
                                    v_sb[:, kc0, vb:vb + DH + 1],
                                    e0[:, kc0 % 2, :], start=(kc0 == 0),
                                    stop=(kc0 == KC - 1))
                        for kc0, e0 in pend:
                            nc.tensor.matmul(
                                pav[ob:ob + DH + 1, :],
                                v_sb[:, kc0, vb:vb + DH + 1],
                                e0[:, kc0 % 2, :], start=(kc0 == 0),
                                stop=(kc0 == KC - 1))
                        zr = scr_s.tile([1, TQ], F32, tag="zr")
                        nc.vector.reciprocal(zr[:], pav[dr:dr + 1, :])
                        zrb = scr_s.tile([1, TQ], BF16, tag="zrb")
                        nc.vector.tensor_copy(zrb[:], zr[:])
                        ps_z = ps_small.tile([P, TQ], F32, tag="misc")
                        nc.tensor.matmul(ps_z[:DH, :], ones_row[:, :DH],
                                         zrb[:], start=True, stop=True)
                        zb = scr_s.tile([DH, TQ], F32, tag="zb")
                        nc.vector.tensor_copy(zb[:], ps_z[:DH, :])
                        stg64 = scr_s.tile([DH, TQ], BF16, tag="stg64")
                        nc.vector.tensor_mul(stg64[:], pav[:DH, :], zb[:])
                        nc.sync.dma_start(attn_sb[lo:lo + DH, m, :],
                                          stg64[:])

            # ---------- Phase C: Wo + residual, LN2, FFN ----------
            with tc.tile_pool(name="sC", bufs=1) as sC, \
                 tc.tile_pool(name="sCw", bufs=3) as sCw, \
                 tc.tile_pool(name="ps_mc", bufs=2, space="PSUM") as ps_mc:
                for m in range(J if 'wo' in phases else 0):
                    ms = slice(m * P, (m + 1) * P)
                    ps = ps_mc.tile([P, TQ], F32, tag="mm")
                    for j in range(J):
                        nc.tensor.matmul(ps[:], wo_w[:, j, ms],
                                         attn_sb[:, j, :],
                                         start=(j == 0), stop=(j == J - 1))
                    nc.vector.tensor_add(y_sb[:, m, :], ps[:], xq_sb[:, m, :])

                ln2_sb = sC.tile([P, J, TQ], BF16)
                if 'ffn' in phases:
                    layer_norm(y_sb, ln2g_c, ln2b_c, ln2_sb, True)

                h1_sb = sC.tile([P, JF, TQ], BF16)
                for mf in range(JF if 'ffn' in phases else 0):
                    mfs = slice(mf * P, (mf + 1) * P)
                    wt = sCw.tile([P, J, P], BF16, tag="w8f", bufs=6)
                    nc.sync.dma_start(wt[:], w1T_r[:, :, mfs])
                    wsrc = wt[:]
                    ps = ps_mc.tile([P, TQ], F32, tag="mm")
                    for j in range(J):
                        nc.tensor.matmul(ps[:], wsrc[:, j, :], ln2_sb[:, j, :],
                                         start=(j == 0), stop=(j == J - 1))
                    nc.scalar.activation(h1_sb[:, mf, :], ps[:], AF.Relu,
                                         bias=b1_c[:, mf:mf + 1], scale=1.0)

                for m in range(J if 'ffn' in phases else 0):
                    ms = slice(m * P, (m + 1) * P)
                    wt = sCw.tile([P, JF, P], BF16, tag="w32")
                    nc.sync.dma_start(wt[:], w2T_r[:, :, ms])
                    ps = ps_mc.tile([P, TQ], F32, tag="mm")
                    for jf in range(JF):
                        nc.tensor.matmul(ps[:], wt[:, jf, :], h1_sb[:, jf, :],
                                         start=(jf == 0), stop=(jf == JF - 1))
                    t3 = sCw.tile([P, TQ], F32, tag="t3", bufs=2)
                    nc.vector.tensor_add(t3[:], ps[:], y_sb[:, m, :])
                    ot = sCw.tile([P, TQ], F32, tag="ot", bufs=2)
                    nc.scalar.activation(ot[:], t3[:], AF.Identity,
                                         bias=b2_c[:, m:m + 1], scale=1.0)
                    nc.sync.dma_start(outT_r[:, m, :], ot[:])

    nc.compile()
    return nc


def _get_program(S, D, H, DFF, TQ, n_iter=1,
                 phases=frozenset({'lnq', 'qkv', 'attn', 'wo', 'ffn'})):
    key = (S, D, H, DFF, TQ, n_iter, phases)
    if key not in _PROG_CACHE:
        _PROG_CACHE[key] = _build_program(S, D, H, DFF, TQ, n_iter, phases)
    return _PROG_CACHE[key]


def _run(x, mask, ln1_g, ln1_b, Wq, Wk, Wv, Wo, ln2_g, ln2_b, W1, b1, W2, b2,
         n_cores, trace=False, n_iter=1):
    B, S, D = x.shape
    DFF = W1.shape[0]
    H = D // DH
    cores_per_b = n_cores // B
    TQ = S // cores_per_b
    NT = S // TQ

    nc = _get_program(S, D, H, DFF, TQ, n_iter)

    bf = ml_dtypes.bfloat16
    f32 = np.float32
    shared = dict(
        wqT=np.ascontiguousarray(np.asarray(Wq, f32).T).astype(bf),
        wkT=np.ascontiguousarray(np.asarray(Wk, f32).T).astype(bf),
        wvT=np.ascontiguousarray(np.asarray(Wv, f32).T).astype(bf),
        woT=np.ascontiguousarray(np.asarray(Wo, f32).T).astype(bf),
        w1T=np.ascontiguousarray(np.asarray(W1, f32).T).astype(bf),
        w2T=np.ascontiguousarray(np.asarray(W2, f32).T).astype(bf),
        ln1g=np.asarray(ln1_g, f32), ln1b=np.asarray(ln1_b, f32),
        ln2g=np.asarray(ln2_g, f32), ln2b=np.asarray(ln2_b, f32),
        b1=np.asarray(b1, f32), b2=np.asarray(b2, f32),
    )
    mask2d = np.asarray(mask).reshape(S, S)  # [q, k] bool
    x = np.asarray(x, f32)

    in_maps = []
    for c in range(n_cores):
        b = c // cores_per_b
        qt = c % cores_per_b
        q0 = qt * TQ
        # permuted context: chunks != qt in order, own chunk last
        perm = [t for t in range(NT) if t != qt] + [qt]
        xb16 = x[b].T.astype(bf)  # [D, S]
        xcT = np.ascontiguousarray(
            np.concatenate([xb16[:, t * TQ:(t + 1) * TQ] for t in perm],
                           axis=1))
        vz = np.concatenate([
            mask2d[q0:q0 + TQ, t * TQ:(t + 1) * TQ].any(axis=0).astype(f32)
            if t != qt else np.ones(TQ, f32)
            for t in perm])
        xqT = np.ascontiguousarray(x[b, q0:q0 + TQ].T)
        mT = np.ascontiguousarray(
            mask2d[q0:q0 + TQ, q0:q0 + TQ].T.astype(f32)).astype(bf)
        in_maps.append(dict(shared, xcT=xcT, xqT=xqT, maskT=mT, vzero=vz))

    res = run_bass_kernel_spmd(nc, in_maps, list(range(n_cores)), trace=trace)

    out = np.empty((B, S, D), f32)
    for c in range(n_cores):
        b = c // cores_per_b
        q0 = (c % cores_per_b) * TQ
        out[b, q0:q0 + TQ, :] = res.results[c]["outT"].T
    return out, res


def kernel(x, mask, ln1_g, ln1_b, Wq, Wk, Wv, Wo, ln2_g, ln2_b, W1, b1, W2,
           b2):
    out, _ = _run(x, mask, ln1_g, ln1_b, Wq, Wk, Wv, Wo, ln2_g, ln2_b,
                  W1, b1, W2, b2, n_cores=8)
    return out


# revision 18
# speedup vs baseline: 1.3458x; 1.3458x over previous
"""Trainium2 Bass kernel for a pre-LN transformer decoder block.

Problem: x:[2,2048,1024] f32, causal mask, 16 heads, DFF=4096.
  out = x + Attn(LN1(x)); out = out + FFN(LN2(out))

Strategy (8 NeuronCores, collective-free SPMD):
  - Core c handles batch c//4, query rows [(c%4)*512, +512).
  - Each core redundantly computes LN1 + K/V over its batch's full 2048
    context (uniform program; per-core differences live only in input
    data).
  - Context PERMUTATION trick: the host reorders each core's context
    chunks so the core's own (diagonal) 512-chunk sits at fixed position
    [1536, 2048). Causal masking then needs only (a) a 512x512 diagonal
    mask applied to the last 4 key tiles, and (b) a per-key "alive"
    column (vzero) folded into V staging, which zeroes both the V rows
    and the softmax-denominator ones-column of dead keys. Attention sums
    are key-order invariant, so results are exact.
  - Everything stays in SBUF: Q/K/V, attention outputs; weights are
    streamed or prefetched from HBM under compute.
  - Softmax without max-subtraction (scores provably small); row sums
    via a ones-column appended to V; exp batched 2 PSUM banks per
    activation instruction.
  - Matmuls bf16 (f32 PSUM accumulate); LN stats in f32/bf16 mix.
"""

import sys
import contextlib
import numpy as np

for _p in ("/opt/trn_rl_repo", "/root/.axon_site/_ro/trn_rl_repo"):
    if _p not in sys.path:
        sys.path.insert(0, _p)

import ml_dtypes  # noqa: E402
import concourse.bass as bass  # noqa: E402
import concourse.mybir as mybir  # noqa: E402
import concourse.tile as tile  # noqa: E402
from concourse import bacc  # noqa: E402
from concourse.bass_utils import run_bass_kernel_spmd  # noqa: E402
from concourse.masks import make_identity  # noqa: E402

P = 128
DH = 64
EPS = 1e-5
BF16 = mybir.dt.bfloat16
F32 = mybir.dt.float32
AF = mybir.ActivationFunctionType

_PROG_CACHE = {}


def _build_program(S, D, H, DFF, TQ, n_iter=1,
                   phases=frozenset({'lnq', 'qkv', 'attn', 'wo', 'ffn'})):
    """One-core SPMD program: decoder block for TQ=512 query rows with a
    (permuted) S=2048-row context. All cores run this identical program
    on different data."""
    J = D // P            # 8 feature tiles
    JF = DFF // P         # 32
    KC = S // P           # 16 key tiles
    NT = S // TQ          # 4 context chunks of 512
    VW = 2 * (DH + 1)     # v_sb cols per head-pair tile (130)
    assert TQ == 512 and H == 2 * J

    nc = bacc.Bacc(None, target_bir_lowering=False)

    # ---- I/O ----
    xcT = nc.dram_tensor("xcT", [D, S], BF16, kind="ExternalInput")
    xqT = nc.dram_tensor("xqT", [D, TQ], F32, kind="ExternalInput")
    maskT = nc.dram_tensor("maskT", [TQ, TQ], BF16, kind="ExternalInput")
    vzero = nc.dram_tensor("vzero", [S], F32, kind="ExternalInput")
    wqT = nc.dram_tensor("wqT", [D, D], BF16, kind="ExternalInput")
    wkT = nc.dram_tensor("wkT", [D, D], BF16, kind="ExternalInput")
    wvT = nc.dram_tensor("wvT", [D, D], BF16, kind="ExternalInput")
    woT = nc.dram_tensor("woT", [D, D], BF16, kind="ExternalInput")
    w1T = nc.dram_tensor("w1T", [D, DFF], BF16, kind="ExternalInput")
    w2T = nc.dram_tensor("w2T", [DFF, D], BF16, kind="ExternalInput")
    ln1g = nc.dram_tensor("ln1g", [D], F32, kind="ExternalInput")
    ln1b = nc.dram_tensor("ln1b", [D], F32, kind="ExternalInput")
    ln2g = nc.dram_tensor("ln2g", [D], F32, kind="ExternalInput")
    ln2b = nc.dram_tensor("ln2b", [D], F32, kind="ExternalInput")
    b1 = nc.dram_tensor("b1", [DFF], F32, kind="ExternalInput")
    b2 = nc.dram_tensor("b2", [D], F32, kind="ExternalInput")
    outT = nc.dram_tensor("outT", [D, TQ], F32, kind="ExternalOutput")

    xcT_r = xcT.rearrange("(j p) t -> p j t", p=P)
    xqT_r = xqT.rearrange("(j p) t -> p j t", p=P)
    maskT_r = maskT.rearrange("(kc p) q -> p kc q", p=P)
    vz_r = vzero.rearrange("(kc p) -> p kc", p=P)
    wqT_r = wqT.rearrange("(j p) e -> p j e", p=P)
    wkT_r = wkT.rearrange("(j p) e -> p j e", p=P)
    wvT_r = wvT.rearrange("(j p) e -> p j e", p=P)
    woT_r = woT.rearrange("(j p) e -> p j e", p=P)
    w1T_r = w1T.rearrange("(j p) f -> p j f", p=P)
    w2T_r = w2T.rearrange("(jf p) e -> p jf e", p=P)
    outT_r = outT.rearrange("(j p) q -> p j q", p=P)

    loop_cm = nc.Fori(0, n_iter) if n_iter > 1 else contextlib.nullcontext()
    with loop_cm, tile.TileContext(nc) as tc:
        with (
            tc.tile_pool(name="const", bufs=1) as const,
            tc.tile_pool(name="persist", bufs=1) as persist,
            tc.tile_pool(name="scr", bufs=2) as scr,
            tc.tile_pool(name="scr_s", bufs=2) as scr_s,
            tc.tile_pool(name="ps_small", bufs=2, space="PSUM") as ps_small,
        ):
            # constants
            ones_col = const.tile([P, 1], BF16)
            nc.vector.memset(ones_col[:], 1.0)
            ones_row = const.tile([1, P], BF16)
            nc.vector.memset(ones_row[:], 1.0)
            eps_t = const.tile([1, 1], F32)
            nc.vector.memset(eps_t[:], EPS)
            ident = const.tile([P, P], BF16)
            make_identity(nc, ident[:])
            ln1g_c = const.tile([P, J], F32)
            nc.sync.dma_start(ln1g_c[:], ln1g.rearrange("(j p) -> p j", p=P))
            ln1b_c = const.tile([P, J], F32)
            nc.sync.dma_start(ln1b_c[:], ln1b.rearrange("(j p) -> p j", p=P))
            ln2g_c = const.tile([P, J], F32)
            nc.sync.dma_start(ln2g_c[:], ln2g.rearrange("(j p) -> p j", p=P))
            ln2b_c = const.tile([P, J], F32)
            nc.sync.dma_start(ln2b_c[:], ln2b.rearrange("(j p) -> p j", p=P))
            b1_c = const.tile([P, JF], F32)
            nc.sync.dma_start(b1_c[:], b1.rearrange("(j p) -> p j", p=P))
            b2_c = const.tile([P, J], F32)
            nc.sync.dma_start(b2_c[:], b2.rearrange("(j p) -> p j", p=P))
            vz_c = const.tile([P, KC], F32)
            nc.sync.dma_start(vz_c[:], vz_r)
            vzb_c = const.tile([P, KC], BF16)
            nc.vector.tensor_copy(vzb_c[:], vz_c[:])

            # persistent activations
            xq_sb = persist.tile([P, J, TQ], F32)
            nc.sync.dma_start(xq_sb[:], xqT_r)
            y_sb = persist.tile([P, J, TQ], F32)
            attn_sb = persist.tile([P, J, TQ], BF16)
            wo_w = persist.tile([P, J, D], BF16)

            def layer_norm(src_sb, g_c, b_c, out_sb, src_f32):
                """Column LN over J*P features: src [P,J,TQ] -> out bf16."""
                ps_x = ps_small.tile([P, TQ], F32, tag="misc")
                ps_q = ps_small.tile([P, TQ], F32, tag="misc")
                xbs = []
                for j in range(J):
                    if src_f32:
                        xb = scr.tile([P, TQ], BF16, tag="xb")
                        nc.vector.tensor_copy(xb[:], src_sb[:, j, :])
                        xb = xb[:]
                    else:
                        xb = src_sb[:, j, :]
                    xbs.append(xb)
                    sq = scr.tile([P, TQ], BF16, tag="sq")
                    nc.vector.tensor_mul(sq[:], xb, xb)
                    nc.tensor.matmul(ps_x[:1, :], ones_col[:], xb,
                                     start=(j == 0), stop=(j == J - 1))
                    nc.tensor.matmul(ps_q[:1, :], ones_col[:], sq[:],
                                     start=(j == 0), stop=(j == J - 1))
                inv_d = 1.0 / D
                mu = scr_s.tile([1, TQ], F32, tag="mu", bufs=1)
                nc.scalar.mul(mu[:], ps_x[:1, :], inv_d)
                ex2 = scr_s.tile([1, TQ], F32, tag="ex2", bufs=1)
                nc.scalar.mul(ex2[:], ps_q[:1, :], inv_d)
                var = scr_s.tile([1, TQ], F32, tag="var", bufs=1)
                nc.vector.tensor_mul(var[:], mu[:], mu[:])
                nc.vector.tensor_sub(var[:], ex2[:], var[:])
                nc.scalar.activation(var[:], var[:], AF.Ln,
                                     bias=eps_t[:], scale=1.0)
                nc.scalar.activation(var[:], var[:], AF.Exp,
                                     bias=0.0, scale=-0.5)
                mub = scr_s.tile([1, TQ], BF16, tag="mub")
                nc.scalar.copy(mub[:], mu[:])
                rsb = scr_s.tile([1, TQ], BF16, tag="rsb")
                nc.scalar.copy(rsb[:], var[:])
                pmu = ps_small.tile([P, TQ], F32, tag="misc")
                nc.tensor.matmul(pmu[:], ones_row[:], mub[:],
                                 start=True, stop=True)
                prs = ps_small.tile([P, TQ], F32, tag="misc")
                nc.tensor.matmul(prs[:], ones_row[:], rsb[:],
                                 start=True, stop=True)
                for j in range(J):
                    t1 = scr.tile([P, TQ], F32, tag="t1")
                    nc.vector.tensor_sub(t1[:], src_sb[:, j, :], pmu[:])
                    nc.vector.tensor_mul(t1[:], t1[:], prs[:])
                    nc.scalar.activation(out_sb[:, j, :], t1[:], AF.Identity,
                                         bias=b_c[:, j:j + 1],
                                         scale=g_c[:, j:j + 1])

            # attention-data pool (lives through attention)
            with tc.tile_pool(name="attd", bufs=1) as attd:
                q_sb = attd.tile([P, J, TQ], BF16)
                k_sb = attd.tile([P, J, S], BF16)
                v_sb = attd.tile([P, KC, J * VW], BF16)
                mask_sb = attd.tile([P, 4, TQ], BF16)
                nc.sync.dma_start(mask_sb[:], maskT_r)
                # ones columns (killed later for dead keys by vzb values)
                for hh in range(H):
                    c0 = (hh // 2) * VW + (hh % 2) * (DH + 1) + DH
                    nc.vector.tensor_copy(v_sb[:, :, c0], vzb_c[:])

                # ---------- Phase A: LN1 + Q/K/V projections ----------
                with tc.tile_pool(name="sA", bufs=1) as sA, \
                     tc.tile_pool(name="sAw", bufs=3) as sAw, \
                     tc.tile_pool(name="sAl", bufs=2) as sAl, \
                     tc.tile_pool(name="ps_mm", bufs=2, space="PSUM") as ps_mm, \
                     tc.tile_pool(name="ps_tr", bufs=2, space="PSUM") as ps_tr:
                    ln1_own = sA.tile([P, J, TQ], BF16)
                    if 'lnq' in phases:
                        layer_norm(xq_sb, ln1g_c, ln1b_c, ln1_own, True)
                    for m in range(J if 'lnq' in phases else 0):
                        ms = slice(m * P, (m + 1) * P)
                        wt = sAw.tile([P, J, P], BF16, tag="w8")
                        nc.sync.dma_start(wt[:], wqT_r[:, :, ms])
                        ps = ps_mm.tile([P, TQ], F32, tag="mm")
                        for j in range(J):
                            nc.tensor.matmul(ps[:], wt[:, j, :],
                                             ln1_own[:, j, :],
                                             start=(j == 0), stop=(j == J - 1))
                        nc.scalar.copy(q_sb[:, m, :], ps[:])

                    for t in range(NT if 'qkv' in phases else 0):
                        ts = slice(t * TQ, (t + 1) * TQ)
                        xct = sAl.tile([P, J, TQ], BF16, tag="xct", bufs=1)
                        nc.sync.dma_start(xct[:], xcT_r[:, :, ts])
                        ln1_t = sAl.tile([P, J, TQ], BF16, tag="ln1")
                        layer_norm(xct, ln1g_c, ln1b_c, ln1_t, False)
                        for m in range(J):
                            ms = slice(m * P, (m + 1) * P)
                            wt = sAw.tile([P, J, P], BF16, tag="w8")
                            nc.sync.dma_start(wt[:], wkT_r[:, :, ms])
                            ps = ps_mm.tile([P, TQ], F32, tag="mm")
                            for j in range(J):
                                nc.tensor.matmul(ps[:], wt[:, j, :],
                                                 ln1_t[:, j, :],
                                                 start=(j == 0),
                                                 stop=(j == J - 1))
                            nc.scalar.copy(k_sb[:, m, ts], ps[:])

                            wt = sAw.tile([P, J, P], BF16, tag="w8")
                            nc.sync.dma_start(wt[:], wvT_r[:, :, ms])
                            ps = ps_mm.tile([P, TQ], F32, tag="mm")
                            for j in range(J):
                                nc.tensor.matmul(ps[:], wt[:, j, :],
                                                 ln1_t[:, j, :],
                                                 start=(j == 0),
                                                 stop=(j == J - 1))
                            vst = scr.tile([P, TQ], BF16, tag="vst")
                            nc.scalar.copy(vst[:], ps[:])
                            for kt in range(4):
                                kc = t * 4 + kt
                                ps_t = ps_tr.tile([P, P], BF16, tag="tr")
                                nc.tensor.transpose(
                                    ps_t[:],
                                    vst[:, kt * P:(kt + 1) * P], ident[:])
                                vb0 = m * VW
                                nc.vector.tensor_scalar_mul(
                                    v_sb[:, kc, vb0:vb0 + DH],
                                    ps_t[:, 0:DH], vz_c[:, kc:kc + 1])
                                nc.vector.tensor_scalar_mul(
                                    v_sb[:, kc, vb0 + DH + 1:vb0 + 2 * DH + 1],
                                    ps_t[:, DH:2 * DH], vz_c[:, kc:kc + 1])

                # ---------- Phase B: attention ----------
                nc.sync.dma_start(wo_w[:], woT_r)
                with tc.tile_pool(name="sBe", bufs=3) as sBe, \
                     tc.tile_pool(name="ps_sc", bufs=2, space="PSUM") as ps_sc, \
                     tc.tile_pool(name="ps_av", bufs=2, space="PSUM") as ps_av:
                    for m in range(J if 'attn' in phases else 0):
                        # both heads of pair m together: scores use PE
                        # quadrant tiles (0,0)/(64,0) concurrently
                        pavs = [ps_av.tile([P, TQ], F32, tag="av",
                                           name=f"pav{m}_{i}")
                                for i in range(2)]
                        pend = []
                        for kc in range(KC):
                            ps2 = ps_sc.tile([P, 2, TQ], F32, tag="sc")
                            nc.tensor.matmul(
                                ps2[:, 0, :],
                                k_sb[0:DH, m, kc * P:(kc + 1) * P],
                                q_sb[0:DH, m, :], start=True, stop=True,
                                tile_position=(0, 0))
                            nc.tensor.matmul(
                                ps2[:, 1, :],
                                k_sb[DH:P, m, kc * P:(kc + 1) * P],
                                q_sb[DH:P, m, :], start=True, stop=True,
                                tile_position=(DH, 0))
                            ex2 = sBe.tile([P, 2, TQ], BF16, tag="ex")
                            nc.scalar.activation(ex2[:], ps2[:], AF.Exp,
                                                 scale=1.0 / np.sqrt(DH))
                            if kc >= KC - 4:
                                nc.vector.tensor_mul(
                                    ex2[:, 0, :], ex2[:, 0, :],
                                    mask_sb[:, kc - (KC - 4), :])
                                nc.vector.tensor_mul(
                                    ex2[:, 1, :], ex2[:, 1, :],
                                    mask_sb[:, kc - (KC - 4), :])
                            pend.append((kc, ex2))
                            while len(pend) > 1:
                                kc0, e0 = pend.pop(0)
                                for h2 in range(2):
                                    vb = m * VW + h2 * (DH + 1)
                                    nc.tensor.matmul(
                                        pavs[h2][:DH + 1, :],
                                        v_sb[:, kc0, vb:vb + DH + 1],
                                        e0[:, h2, :], start=(kc0 == 0),
                                        stop=(kc0 == KC - 1))
                        for kc0, e0 in pend:
                            for h2 in range(2):
                                vb = m * VW + h2 * (DH + 1)
                                nc.tensor.matmul(
                                    pavs[h2][:DH + 1, :],
                                    v_sb[:, kc0, vb:vb + DH + 1],
                                    e0[:, h2, :], start=(kc0 == 0),
                                    stop=(kc0 == KC - 1))
                        for h2 in range(2):
                            lo = h2 * DH
                            pav = pavs[h2]
                            zr = scr_s.tile([1, TQ], F32, tag="zr")
                            nc.vector.reciprocal(zr[:], pav[DH:DH + 1, :])
                            zrb = scr_s.tile([1, TQ], BF16, tag="zrb")
                            nc.vector.tensor_copy(zrb[:], zr[:])
                            ps_z = ps_small.tile([P, TQ], F32, tag="misc")
                            nc.tensor.matmul(ps_z[:DH, :], ones_row[:, :DH],
                                             zrb[:], start=True, stop=True)
                            zb = scr_s.tile([DH, TQ], F32, tag="zb")
                            nc.vector.tensor_copy(zb[:], ps_z[:DH, :])
                            stg64 = scr_s.tile([DH, TQ], BF16, tag="stg64")
                            nc.vector.tensor_mul(stg64[:], pav[:DH, :], zb[:])
                            nc.sync.dma_start(attn_sb[lo:lo + DH, m, :],
                                              stg64[:])

            # ---------- Phase C: Wo + residual, LN2, FFN ----------
            with tc.tile_pool(name="sC", bufs=1) as sC, \
                 tc.tile_pool(name="sCw", bufs=3) as sCw, \
                 tc.tile_pool(name="ps_mc", bufs=2, space="PSUM") as ps_mc:
                for m in range(J if 'wo' in phases else 0):
                    ms = slice(m * P, (m + 1) * P)
                    ps = ps_mc.tile([P, TQ], F32, tag="mm")
                    for j in range(J):
                        nc.tensor.matmul(ps[:], wo_w[:, j, ms],
                                         attn_sb[:, j, :],
                                         start=(j == 0), stop=(j == J - 1))
                    nc.vector.tensor_add(y_sb[:, m, :], ps[:], xq_sb[:, m, :])

                ln2_sb = sC.tile([P, J, TQ], BF16)
                if 'ffn' in phases:
                    layer_norm(y_sb, ln2g_c, ln2b_c, ln2_sb, True)

                h1_sb = sC.tile([P, JF, TQ], BF16)
                for mf in range(JF if 'ffn' in phases else 0):
                    mfs = slice(mf * P, (mf + 1) * P)
                    wt = sCw.tile([P, J, P], BF16, tag="w8f", bufs=6)
                    nc.sync.dma_start(wt[:], w1T_r[:, :, mfs])
                    wsrc = wt[:]
                    ps = ps_mc.tile([P, TQ], F32, tag="mm")
                    for j in range(J):
                        nc.tensor.matmul(ps[:], wsrc[:, j, :], ln2_sb[:, j, :],
                                         start=(j == 0), stop=(j == J - 1))
                    nc.scalar.activation(h1_sb[:, mf, :], ps[:], AF.Relu,
                                         bias=b1_c[:, mf:mf + 1], scale=1.0)

                for m in range(J if 'ffn' in phases else 0):
                    ms = slice(m * P, (m + 1) * P)
                    wt = sCw.tile([P, JF, P], BF16, tag="w32")
                    nc.sync.dma_start(wt[:], w2T_r[:, :, ms])
                    ps = ps_mc.tile([P, TQ], F32, tag="mm")
                    for jf in range(JF):
                        nc.tensor.matmul(ps[:], wt[:, jf, :], h1_sb[:, jf, :],
                                         start=(jf == 0), stop=(jf == JF - 1))
                    t3 = sCw.tile([P, TQ], F32, tag="t3", bufs=2)
                    nc.vector.tensor_add(t3[:], ps[:], y_sb[:, m, :])
                    ot = sCw.tile([P, TQ], F32, tag="ot", bufs=2)
                    nc.scalar.activation(ot[:], t3[:], AF.Identity,
                                         bias=b2_c[:, m:m + 1], scale=1.0)
                    nc.sync.dma_start(outT_r[:, m, :], ot[:])

    nc.compile()
    return nc


def _get_program(S, D, H, DFF, TQ, n_iter=1,
                 phases=frozenset({'lnq', 'qkv', 'attn', 'wo', 'ffn'})):
    key = (S, D, H, DFF, TQ, n_iter, phases)
    if key not in _PROG_CACHE:
        _PROG_CACHE[key] = _build_program(S, D, H, DFF, TQ, n_iter, phases)
    return _PROG_CACHE[key]


def _run(x, mask, ln1_g, ln1_b, Wq, Wk, Wv, Wo, ln2_g, ln2_b, W1, b1, W2, b2,
         n_cores, trace=False, n_iter=1):
    B, S, D = x.shape
    DFF = W1.shape[0]
    H = D // DH
    cores_per_b = n_cores // B
    TQ = S // cores_per_b
    NT = S // TQ

    nc = _get_program(S, D, H, DFF, TQ, n_iter)

    bf = ml_dtypes.bfloat16
    f32 = np.float32
    shared = dict(
        wqT=np.ascontiguousarray(np.asarray(Wq, f32).T).astype(bf),
        wkT=np.ascontiguousarray(np.asarray(Wk, f32).T).astype(bf),
        wvT=np.ascontiguousarray(np.asarray(Wv, f32).T).astype(bf),
        woT=np.ascontiguousarray(np.asarray(Wo, f32).T).astype(bf),
        w1T=np.ascontiguousarray(np.asarray(W1, f32).T).astype(bf),
        w2T=np.ascontiguousarray(np.asarray(W2, f32).T).astype(bf),
        ln1g=np.asarray(ln1_g, f32), ln1b=np.asarray(ln1_b, f32),
        ln2g=np.asarray(ln2_g, f32), ln2b=np.asarray(ln2_b, f32),
        b1=np.asarray(b1, f32), b2=np.asarray(b2, f32),
    )
    mask2d = np.asarray(mask).reshape(S, S)  # [q, k] bool
    x = np.asarray(x, f32)

    in_maps = []
    for c in range(n_cores):
        b = c // cores_per_b
        qt = c % cores_per_b
        q0 = qt * TQ
        # permuted context: chunks != qt in order, own chunk last
        perm = [t for t in range(NT) if t != qt] + [qt]
        xb16 = x[b].T.astype(bf)  # [D, S]
        xcT = np.ascontiguousarray(
            np.concatenate([xb16[:, t * TQ:(t + 1) * TQ] for t in perm],
                           axis=1))
        vz = np.concatenate([
            mask2d[q0:q0 + TQ, t * TQ:(t + 1) * TQ].any(axis=0).astype(f32)
            if t != qt else np.ones(TQ, f32)
            for t in perm])
        xqT = np.ascontiguousarray(x[b, q0:q0 + TQ].T)
        mT = np.ascontiguousarray(
            mask2d[q0:q0 + TQ, q0:q0 + TQ].T.astype(f32)).astype(bf)
        in_maps.append(dict(shared, xcT=xcT, xqT=xqT, maskT=mT, vzero=vz))

    res = run_bass_kernel_spmd(nc, in_maps, list(range(n_cores)), trace=trace)

    out = np.empty((B, S, D), f32)
    for c in range(n_cores):
        b = c // cores_per_b
        q0 = (c % cores_per_b) * TQ
        out[b, q0:q0 + TQ, :] = res.results[c]["outT"].T
    return out, res


def kernel(x, mask, ln1_g, ln1_b, Wq, Wk, Wv, Wo, ln2_g, ln2_b, W1, b1, W2,
           b2):
    out, _ = _run(x, mask, ln1_g, ln1_b, Wq, Wk, Wv, Wo, ln2_g, ln2_b,
                  W1, b1, W2, b2, n_cores=8)
    return out


# revision 19
# speedup vs baseline: 1.4459x; 1.0744x over previous
"""Trainium2 Bass kernel for a pre-LN transformer decoder block.

Problem: x:[2,2048,1024] f32, causal mask, 16 heads, DFF=4096.
  out = x + Attn(LN1(x)); out = out + FFN(LN2(out))

Strategy (8 NeuronCores, collective-free SPMD):
  - Core c handles batch c//4, query rows [(c%4)*512, +512).
  - Each core redundantly computes LN1 + K/V over its batch's full 2048
    context (uniform program; per-core differences live only in input
    data).
  - Context PERMUTATION trick: the host reorders each core's context
    chunks so the core's own (diagonal) 512-chunk sits at fixed position
    [1536, 2048). Causal masking then needs only (a) a 512x512 diagonal
    mask applied to the last 4 key tiles, and (b) a per-key "alive"
    column (vzero) folded into V staging, which zeroes both the V rows
    and the softmax-denominator ones-column of dead keys. Attention sums
    are key-order invariant, so results are exact.
  - Everything stays in SBUF: Q/K/V, attention outputs; weights are
    streamed or prefetched from HBM under compute.
  - Softmax without max-subtraction (scores provably small); row sums
    via a ones-column appended to V; exp batched 2 PSUM banks per
    activation instruction.
  - Matmuls bf16 (f32 PSUM accumulate); LN stats in f32/bf16 mix.
"""

import sys
import contextlib
import numpy as np

for _p in ("/opt/trn_rl_repo", "/root/.axon_site/_ro/trn_rl_repo"):
    if _p not in sys.path:
        sys.path.insert(0, _p)

import ml_dtypes  # noqa: E402
import concourse.bass as bass  # noqa: E402
import concourse.mybir as mybir  # noqa: E402
import concourse.tile as tile  # noqa: E402
from concourse import bacc  # noqa: E402
from concourse.bass_utils import run_bass_kernel_spmd  # noqa: E402
from concourse.masks import make_identity  # noqa: E402

P = 128
DH = 64
EPS = 1e-5
BF16 = mybir.dt.bfloat16
F32 = mybir.dt.float32
AF = mybir.ActivationFunctionType

_PROG_CACHE = {}


def _build_program(S, D, H, DFF, TQ, n_iter=1,
                   phases=frozenset({'lnq', 'qkv', 'attn', 'wo', 'ffn'})):
    """One-core SPMD program: decoder block for TQ=512 query rows with a
    (permuted) S=2048-row context. All cores run this identical program
    on different data."""
    J = D // P            # 8 feature tiles
    JF = DFF // P         # 32
    KC = S // P           # 16 key tiles
    NT = S // TQ          # 4 context chunks of 512
    VW = 2 * (DH + 1)     # v_sb cols per head-pair tile (130)
    assert TQ == 512 and H == 2 * J

    nc = bacc.Bacc(None, target_bir_lowering=False)

    # ---- I/O ----
    xcT = nc.dram_tensor("xcT", [D, S], BF16, kind="ExternalInput")
    xqT = nc.dram_tensor("xqT", [D, TQ], F32, kind="ExternalInput")
    maskT = nc.dram_tensor("maskT", [TQ, TQ], BF16, kind="ExternalInput")
    vzero = nc.dram_tensor("vzero", [S], F32, kind="ExternalInput")
    wqT = nc.dram_tensor("wqT", [D, D], BF16, kind="ExternalInput")
    wkT = nc.dram_tensor("wkT", [D, D], BF16, kind="ExternalInput")
    wvT = nc.dram_tensor("wvT", [D, D], BF16, kind="ExternalInput")
    woT = nc.dram_tensor("woT", [D, D], BF16, kind="ExternalInput")
    w1T = nc.dram_tensor("w1T", [D, DFF], BF16, kind="ExternalInput")
    w2T = nc.dram_tensor("w2T", [DFF, D], BF16, kind="ExternalInput")
    ln1g = nc.dram_tensor("ln1g", [D], F32, kind="ExternalInput")
    ln1b = nc.dram_tensor("ln1b", [D], F32, kind="ExternalInput")
    ln2g = nc.dram_tensor("ln2g", [D], F32, kind="ExternalInput")
    ln2b = nc.dram_tensor("ln2b", [D], F32, kind="ExternalInput")
    b1 = nc.dram_tensor("b1", [DFF], F32, kind="ExternalInput")
    b2 = nc.dram_tensor("b2", [D], F32, kind="ExternalInput")
    outT = nc.dram_tensor("outT", [D, TQ], F32, kind="ExternalOutput")

    xcT_r = xcT.rearrange("(j p) t -> p j t", p=P)
    xqT_r = xqT.rearrange("(j p) t -> p j t", p=P)
    maskT_r = maskT.rearrange("(kc p) q -> p kc q", p=P)
    vz_r = vzero.rearrange("(kc p) -> p kc", p=P)
    wqT_r = wqT.rearrange("(j p) e -> p j e", p=P)
    wkT_r = wkT.rearrange("(j p) e -> p j e", p=P)
    wvT_r = wvT.rearrange("(j p) e -> p j e", p=P)
    woT_r = woT.rearrange("(j p) e -> p j e", p=P)
    w1T_r = w1T.rearrange("(j p) f -> p j f", p=P)
    w2T_r = w2T.rearrange("(jf p) e -> p jf e", p=P)
    outT_r = outT.rearrange("(j p) q -> p j q", p=P)

    loop_cm = nc.Fori(0, n_iter) if n_iter > 1 else contextlib.nullcontext()
    with loop_cm, tile.TileContext(nc) as tc:
        with (
            tc.tile_pool(name="const", bufs=1) as const,
            tc.tile_pool(name="persist", bufs=1) as persist,
            tc.tile_pool(name="scr", bufs=2) as scr,
            tc.tile_pool(name="scr_s", bufs=2) as scr_s,
            tc.tile_pool(name="ps_small", bufs=2, space="PSUM") as ps_small,
        ):
            # constants
            ones_col = const.tile([P, 1], BF16)
            nc.vector.memset(ones_col[:], 1.0)
            ones_row = const.tile([1, P], BF16)
            nc.vector.memset(ones_row[:], 1.0)
            eps_t = const.tile([1, 1], F32)
            nc.vector.memset(eps_t[:], EPS)
            ident = const.tile([P, P], BF16)
            make_identity(nc, ident[:])
            ln1g_c = const.tile([P, J], F32)
            nc.sync.dma_start(ln1g_c[:], ln1g.rearrange("(j p) -> p j", p=P))
            ln1b_c = const.tile([P, J], F32)
            nc.sync.dma_start(ln1b_c[:], ln1b.rearrange("(j p) -> p j", p=P))
            ln2g_c = const.tile([P, J], F32)
            nc.sync.dma_start(ln2g_c[:], ln2g.rearrange("(j p) -> p j", p=P))
            ln2b_c = const.tile([P, J], F32)
            nc.sync.dma_start(ln2b_c[:], ln2b.rearrange("(j p) -> p j", p=P))
            b1_c = const.tile([P, JF], F32)
            nc.sync.dma_start(b1_c[:], b1.rearrange("(j p) -> p j", p=P))
            b2_c = const.tile([P, J], F32)
            nc.sync.dma_start(b2_c[:], b2.rearrange("(j p) -> p j", p=P))
            vz_c = const.tile([P, KC], F32)
            nc.sync.dma_start(vz_c[:], vz_r)
            vzb_c = const.tile([P, KC], BF16)
            nc.vector.tensor_copy(vzb_c[:], vz_c[:])

            # persistent activations
            xq_sb = persist.tile([P, J, TQ], F32)
            nc.sync.dma_start(xq_sb[:], xqT_r)
            y_sb = persist.tile([P, J, TQ], F32)
            attn_sb = persist.tile([P, J, TQ], BF16)
            wo_w = persist.tile([P, J, D], BF16)

            def layer_norm(src_sb, g_c, b_c, out_sb, src_f32):
                """Column LN over J*P features: src [P,J,TQ] -> out bf16."""
                ps_x = ps_small.tile([P, TQ], F32, tag="misc")
                ps_q = ps_small.tile([P, TQ], F32, tag="misc")
                xbs = []
                for j in range(J):
                    if src_f32:
                        xb = scr.tile([P, TQ], BF16, tag="xb")
                        nc.vector.tensor_copy(xb[:], src_sb[:, j, :])
                        xb = xb[:]
                    else:
                        xb = src_sb[:, j, :]
                    xbs.append(xb)
                    sq = scr.tile([P, TQ], BF16, tag="sq")
                    nc.vector.tensor_mul(sq[:], xb, xb)
                    nc.tensor.matmul(ps_x[:1, :], ones_col[:], xb,
                                     start=(j == 0), stop=(j == J - 1))
                    nc.tensor.matmul(ps_q[:1, :], ones_col[:], sq[:],
                                     start=(j == 0), stop=(j == J - 1))
                inv_d = 1.0 / D
                mu = scr_s.tile([1, TQ], F32, tag="mu", bufs=1)
                nc.scalar.mul(mu[:], ps_x[:1, :], inv_d)
                ex2 = scr_s.tile([1, TQ], F32, tag="ex2", bufs=1)
                nc.scalar.mul(ex2[:], ps_q[:1, :], inv_d)
                var = scr_s.tile([1, TQ], F32, tag="var", bufs=1)
                nc.vector.tensor_mul(var[:], mu[:], mu[:])
                nc.vector.tensor_sub(var[:], ex2[:], var[:])
                nc.scalar.activation(var[:], var[:], AF.Ln,
                                     bias=eps_t[:], scale=1.0)
                nc.scalar.activation(var[:], var[:], AF.Exp,
                                     bias=0.0, scale=-0.5)
                mub = scr_s.tile([1, TQ], BF16, tag="mub")
                nc.scalar.copy(mub[:], mu[:])
                rsb = scr_s.tile([1, TQ], BF16, tag="rsb")
                nc.scalar.copy(rsb[:], var[:])
                pmu = ps_small.tile([P, TQ], F32, tag="misc")
                nc.tensor.matmul(pmu[:], ones_row[:], mub[:],
                                 start=True, stop=True)
                prs = ps_small.tile([P, TQ], F32, tag="misc")
                nc.tensor.matmul(prs[:], ones_row[:], rsb[:],
                                 start=True, stop=True)
                for j in range(J):
                    t1 = scr.tile([P, TQ], F32, tag="t1")
                    nc.vector.tensor_sub(t1[:], src_sb[:, j, :], pmu[:])
                    nc.vector.tensor_mul(t1[:], t1[:], prs[:])
                    nc.scalar.activation(out_sb[:, j, :], t1[:], AF.Identity,
                                         bias=b_c[:, j:j + 1],
                                         scale=g_c[:, j:j + 1])

            # attention-data pool (lives through attention)
            with tc.tile_pool(name="attd", bufs=1) as attd:
                q_sb = attd.tile([P, J, TQ], BF16)
                k_sb = attd.tile([P, J, S], BF16)
                v_sb = attd.tile([P, KC, J * VW], BF16)
                mask_sb = attd.tile([P, 4, TQ], BF16)
                nc.sync.dma_start(mask_sb[:], maskT_r)
                # ones columns (killed later for dead keys by vzb values)
                for hh in range(H):
                    c0 = (hh // 2) * VW + (hh % 2) * (DH + 1) + DH
                    nc.vector.tensor_copy(v_sb[:, :, c0], vzb_c[:])

                # ---------- Phase A: LN1 + Q/K/V projections ----------
                with tc.tile_pool(name="sA", bufs=1) as sA, \
                     tc.tile_pool(name="sAw", bufs=3) as sAw, \
                     tc.tile_pool(name="sAl", bufs=2) as sAl, \
                     tc.tile_pool(name="ps_mm", bufs=2, space="PSUM") as ps_mm, \
                     tc.tile_pool(name="ps_tr", bufs=2, space="PSUM") as ps_tr:
                    ln1_own = sA.tile([P, J, TQ], BF16)
                    if 'lnq' in phases:
                        layer_norm(xq_sb, ln1g_c, ln1b_c, ln1_own, True)
                    for m in range(J if 'lnq' in phases else 0):
                        ms = slice(m * P, (m + 1) * P)
                        wt = sAw.tile([P, J, P], BF16, tag="w8")
                        nc.sync.dma_start(wt[:], wqT_r[:, :, ms])
                        ps = ps_mm.tile([P, TQ], F32, tag="mm")
                        for j in range(J):
                            nc.tensor.matmul(ps[:], wt[:, j, :],
                                             ln1_own[:, j, :],
                                             start=(j == 0), stop=(j == J - 1))
                        nc.scalar.copy(q_sb[:, m, :], ps[:])

                    for t in range(NT if 'qkv' in phases else 0):
                        ts = slice(t * TQ, (t + 1) * TQ)
                        xct = sAl.tile([P, J, TQ], BF16, tag="xct", bufs=1)
                        nc.sync.dma_start(xct[:], xcT_r[:, :, ts])
                        ln1_t = sAl.tile([P, J, TQ], BF16, tag="ln1")
                        layer_norm(xct, ln1g_c, ln1b_c, ln1_t, False)
                        for m in range(J):
                            ms = slice(m * P, (m + 1) * P)
                            wt = sAw.tile([P, J, P], BF16, tag="w8")
                            nc.sync.dma_start(wt[:], wkT_r[:, :, ms])
                            ps = ps_mm.tile([P, TQ], F32, tag="mm")
                            for j in range(J):
                                nc.tensor.matmul(ps[:], wt[:, j, :],
                                                 ln1_t[:, j, :],
                                                 start=(j == 0),
                                                 stop=(j == J - 1))
                            nc.scalar.copy(k_sb[:, m, ts], ps[:])

                            wt = sAw.tile([P, J, P], BF16, tag="w8")
                            nc.sync.dma_start(wt[:], wvT_r[:, :, ms])
                            ps = ps_mm.tile([P, TQ], F32, tag="mm")
                            for j in range(J):
                                nc.tensor.matmul(ps[:], wt[:, j, :],
                                                 ln1_t[:, j, :],
                                                 start=(j == 0),
                                                 stop=(j == J - 1))
                            vst = scr.tile([P, TQ], BF16, tag="vst")
                            nc.scalar.copy(vst[:], ps[:])
                            for kt in range(4):
                                kc = t * 4 + kt
                                ps_t = ps_tr.tile([P, P], BF16, tag="tr")
                                nc.tensor.transpose(
                                    ps_t[:],
                                    vst[:, kt * P:(kt + 1) * P], ident[:])
                                vb0 = m * VW
                                nc.vector.tensor_scalar_mul(
                                    v_sb[:, kc, vb0:vb0 + DH],
                                    ps_t[:, 0:DH], vz_c[:, kc:kc + 1])
                                nc.vector.tensor_scalar_mul(
                                    v_sb[:, kc, vb0 + DH + 1:vb0 + 2 * DH + 1],
                                    ps_t[:, DH:2 * DH], vz_c[:, kc:kc + 1])

                # ---------- Phase B: attention ----------
                nc.sync.dma_start(wo_w[:], woT_r)
                with tc.tile_pool(name="sBe", bufs=4) as sBe, \
                     tc.tile_pool(name="ps_sc", bufs=2, space="PSUM") as ps_sc, \
                     tc.tile_pool(name="ps_av", bufs=2, space="PSUM") as ps_av:
                    for m in range(J if 'attn' in phases else 0):
                        # both heads of pair m together: scores use PE
                        # quadrant tiles (0,0)/(64,0) concurrently
                        pavs = [ps_av.tile([P, TQ], F32, tag="av",
                                           name=f"pav{m}_{i}")
                                for i in range(2)]
                        pend = []
                        for kc in range(KC):
                            ps2 = ps_sc.tile([P, 2, TQ], F32, tag="sc")
                            nc.tensor.matmul(
                                ps2[:, 0, :],
                                k_sb[0:DH, m, kc * P:(kc + 1) * P],
                                q_sb[0:DH, m, :], start=True, stop=True,
                                tile_position=(0, 0))
                            nc.tensor.matmul(
                                ps2[:, 1, :],
                                k_sb[DH:P, m, kc * P:(kc + 1) * P],
                                q_sb[DH:P, m, :], start=True, stop=True,
                                tile_position=(DH, 0))
                            ex2 = sBe.tile([P, 2, TQ], BF16, tag="ex")
                            nc.scalar.activation(ex2[:], ps2[:], AF.Exp,
                                                 scale=1.0 / np.sqrt(DH))
                            if kc >= KC - 4:
                                nc.vector.tensor_mul(
                                    ex2[:, 0, :], ex2[:, 0, :],
                                    mask_sb[:, kc - (KC - 4), :])
                                nc.vector.tensor_mul(
                                    ex2[:, 1, :], ex2[:, 1, :],
                                    mask_sb[:, kc - (KC - 4), :])
                            pend.append((kc, ex2))
                            while len(pend) > 2:
                                kc0, e0 = pend.pop(0)
                                for h2 in range(2):
                                    vb = m * VW + h2 * (DH + 1)
                                    nc.tensor.matmul(
                                        pavs[h2][:DH + 1, :],
                                        v_sb[:, kc0, vb:vb + DH + 1],
                                        e0[:, h2, :], start=(kc0 == 0),
                                        stop=(kc0 == KC - 1))
                        for kc0, e0 in pend:
                            for h2 in range(2):
                                vb = m * VW + h2 * (DH + 1)
                                nc.tensor.matmul(
                                    pavs[h2][:DH + 1, :],
                                    v_sb[:, kc0, vb:vb + DH + 1],
                                    e0[:, h2, :], start=(kc0 == 0),
                                    stop=(kc0 == KC - 1))
                        for h2 in range(2):
                            lo = h2 * DH
                            pav = pavs[h2]
                            zr = scr_s.tile([1, TQ], F32, tag="zr")
                            nc.vector.reciprocal(zr[:], pav[DH:DH + 1, :])
                            zrb = scr_s.tile([1, TQ], BF16, tag="zrb")
                            nc.vector.tensor_copy(zrb[:], zr[:])
                            ps_z = ps_small.tile([P, TQ], F32, tag="misc")
                            nc.tensor.matmul(ps_z[:DH, :], ones_row[:, :DH],
                                             zrb[:], start=True, stop=True)
                            zb = scr_s.tile([DH, TQ], F32, tag="zb")
                            nc.vector.tensor_copy(zb[:], ps_z[:DH, :])
                            stg64 = scr_s.tile([DH, TQ], BF16, tag="stg64")
                            nc.vector.tensor_mul(stg64[:], pav[:DH, :], zb[:])
                            nc.sync.dma_start(attn_sb[lo:lo + DH, m, :],
                                              stg64[:])

            # ---------- Phase C: Wo + residual, LN2, FFN ----------
            with tc.tile_pool(name="sC", bufs=1) as sC, \
                 tc.tile_pool(name="sCw", bufs=3) as sCw, \
                 tc.tile_pool(name="ps_mc", bufs=2, space="PSUM") as ps_mc:
                for m in range(J if 'wo' in phases else 0):
                    ms = slice(m * P, (m + 1) * P)
                    ps = ps_mc.tile([P, TQ], F32, tag="mm")
                    for j in range(J):
                        nc.tensor.matmul(ps[:], wo_w[:, j, ms],
                                         attn_sb[:, j, :],
                                         start=(j == 0), stop=(j == J - 1))
                    nc.vector.tensor_add(y_sb[:, m, :], ps[:], xq_sb[:, m, :])

                ln2_sb = sC.tile([P, J, TQ], BF16)
                if 'ffn' in phases:
                    layer_norm(y_sb, ln2g_c, ln2b_c, ln2_sb, True)

                h1_sb = sC.tile([P, JF, TQ], BF16)
                for mf in range(JF if 'ffn' in phases else 0):
                    mfs = slice(mf * P, (mf + 1) * P)
                    wt = sCw.tile([P, J, P], BF16, tag="w8f", bufs=6)
                    nc.sync.dma_start(wt[:], w1T_r[:, :, mfs])
                    wsrc = wt[:]
                    ps = ps_mc.tile([P, TQ], F32, tag="mm")
                    for j in range(J):
                        nc.tensor.matmul(ps[:], wsrc[:, j, :], ln2_sb[:, j, :],
                                         start=(j == 0), stop=(j == J - 1))
                    nc.scalar.activation(h1_sb[:, mf, :], ps[:], AF.Relu,
                                         bias=b1_c[:, mf:mf + 1], scale=1.0)

                for m in range(J if 'ffn' in phases else 0):
                    ms = slice(m * P, (m + 1) * P)
                    wt = sCw.tile([P, JF, P], BF16, tag="w32")
                    nc.sync.dma_start(wt[:], w2T_r[:, :, ms])
                    ps = ps_mc.tile([P, TQ], F32, tag="mm")
                    for jf in range(JF):
                        nc.tensor.matmul(ps[:], wt[:, jf, :], h1_sb[:, jf, :],
                                         start=(jf == 0), stop=(jf == JF - 1))
                    t3 = sCw.tile([P, TQ], F32, tag="t3", bufs=2)
                    nc.vector.tensor_add(t3[:], ps[:], y_sb[:, m, :])
                    ot = sCw.tile([P, TQ], F32, tag="ot", bufs=2)
                    nc.scalar.activation(ot[:], t3[:], AF.Identity,
                                         bias=b2_c[:, m:m + 1], scale=1.0)
                    nc.sync.dma_start(outT_r[:, m, :], ot[:])

    nc.compile()
    return nc


def _get_program(S, D, H, DFF, TQ, n_iter=1,
                 phases=frozenset({'lnq', 'qkv', 'attn', 'wo', 'ffn'})):
    key = (S, D, H, DFF, TQ, n_iter, phases)
    if key not in _PROG_CACHE:
        _PROG_CACHE[key] = _build_program(S, D, H, DFF, TQ, n_iter, phases)
    return _PROG_CACHE[key]


def _run(x, mask, ln1_g, ln1_b, Wq, Wk, Wv, Wo, ln2_g, ln2_b, W1, b1, W2, b2,
         n_cores, trace=False, n_iter=1):
    B, S, D = x.shape
    DFF = W1.shape[0]
    H = D // DH
    cores_per_b = n_cores // B
    TQ = S // cores_per_b
    NT = S // TQ

    nc = _get_program(S, D, H, DFF, TQ, n_iter)

    bf = ml_dtypes.bfloat16
    f32 = np.float32
    shared = dict(
        wqT=np.ascontiguousarray(np.asarray(Wq, f32).T).astype(bf),
        wkT=np.ascontiguousarray(np.asarray(Wk, f32).T).astype(bf),
        wvT=np.ascontiguousarray(np.asarray(Wv, f32).T).astype(bf),
        woT=np.ascontiguousarray(np.asarray(Wo, f32).T).astype(bf),
        w1T=np.ascontiguousarray(np.asarray(W1, f32).T).astype(bf),
        w2T=np.ascontiguousarray(np.asarray(W2, f32).T).astype(bf),
        ln1g=np.asarray(ln1_g, f32), ln1b=np.asarray(ln1_b, f32),
        ln2g=np.asarray(ln2_g, f32), ln2b=np.asarray(ln2_b, f32),
        b1=np.asarray(b1, f32), b2=np.asarray(b2, f32),
    )
    mask2d = np.asarray(mask).reshape(S, S)  # [q, k] bool
    x = np.asarray(x, f32)

    in_maps = []
    for c in range(n_cores):
        b = c // cores_per_b
        qt = c % cores_per_b
        q0 = qt * TQ
        # permuted context: chunks != qt in order, own chunk last
        perm = [t for t in range(NT) if t != qt] + [qt]
        xb16 = x[b].T.astype(bf)  # [D, S]
        xcT = np.ascontiguousarray(
            np.concatenate([xb16[:, t * TQ:(t + 1) * TQ] for t in perm],
                           axis=1))
        vz = np.concatenate([
            mask2d[q0:q0 + TQ, t * TQ:(t + 1) * TQ].any(axis=0).astype(f32)
            if t != qt else np.ones(TQ, f32)
            for t in perm])
        xqT = np.ascontiguousarray(x[b, q0:q0 + TQ].T)
        mT = np.ascontiguousarray(
            mask2d[q0:q0 + TQ, q0:q0 + TQ].T.astype(f32)).astype(bf)
        in_maps.append(dict(shared, xcT=xcT, xqT=xqT, maskT=mT, vzero=vz))

    res = run_bass_kernel_spmd(nc, in_maps, list(range(n_cores)), trace=trace)

    out = np.empty((B, S, D), f32)
    for c in range(n_cores):
        b = c // cores_per_b
        q0 = (c % cores_per_b) * TQ
        out[b, q0:q0 + TQ, :] = res.results[c]["outT"].T
    return out, res


def kernel(x, mask, ln1_g, ln1_b, Wq, Wk, Wv, Wo, ln2_g, ln2_b, W1, b1, W2,
           b2):
    out, _ = _run(x, mask, ln1_g, ln1_b, Wq, Wk, Wv, Wo, ln2_g, ln2_b,
                  W1, b1, W2, b2, n_cores=8)
    return out
